# revision 3
# baseline (speedup 1.0000x reference)
"""Trainium2 Bass kernel for MultiHeadedAttentionBlur.

Math: qkv = x @ W_in^T (chunks v,k,q); per-head logits = SCALE * q @ k^T;
logits' key axis viewed as a 32x32 grid gets a 5x5 reflect-padded gaussian
blur; softmax over keys.

Key identity used here: the blur is linear over the key axis, so
blur(q @ k^T) = q @ (M @ k)^T with M = Bm (x) Bm the 1024x1024 blur matrix
(kron of two 32x32 1-D reflect-blur matrices). Blurring k (S x 64 per head)
instead of logits (S x S per head) removes ~16x of the blur FLOPs, and the
M-matmul doubles as the transpose that the logits matmul needs.

Sharding: data-parallel over batch (B=8 -> 8 cores). Each core computes its
batch element's 12 heads: [12, 1024, 1024] slice of the full [96, 1024, 1024]
output.

Per-core device pipeline (all matmuls in float32r = full-rate fp32):
  1. k  = x @ Wk^T            -> [s, f] layout (s on partitions)
  2. qT = Wq @ x^T            -> [f, s] layout (f on partitions)
  3. kbT = (M @ k)^T via matmul(lhsT=k, rhs=M^T) -> [f, s] layout
  4. per head h, per 128-query chunk: logits = qT_h^T @ kbT_h (K=64),
     exp via ScalarE (scale=0.125 folded in, accum_out = row sums),
     normalize via VectorE reciprocal + tensor_scalar_mul, DMA out.
"""

import numpy as np

S = 1024
E = 768
H = 12
D = 64
B = 8
NE = E // 128  # 6 e-tiles (contraction tiles of the projection)
NS = S // 128  # 8 s-tiles
NF = E // 128  # 6 f-tiles (output feature tiles; 2 heads per tile)
SCALE = 0.125
N_CORES = 8
KSIZE = 5
SIGMA = 1.0
GRID = 32


def _blur_matrix_1d():
    # Matches reference gaussian_kernel1d (fp32) + 'reflect' padding.
    x = (np.arange(KSIZE, dtype=np.float32) - (KSIZE - 1) / 2.0).astype(np.float32)
    g = np.exp(-0.5 * (x / SIGMA) ** 2).astype(np.float32)
    g = (g / g.sum()).astype(np.float32)
    pad = KSIZE // 2
    Bm = np.zeros((GRID, GRID), dtype=np.float32)
    for i in range(GRID):
        for t in range(-pad, pad + 1):
            j = i + t
            if j < 0:
                j = -j
            elif j > GRID - 1:
                j = 2 * (GRID - 1) - j
            Bm[i, j] += g[t + pad]
    return Bm


def _build():
    import concourse.bacc as bacc
    import concourse.mybir as mybir
    import concourse.tile as tile

    f32 = mybir.dt.float32
    f32r = mybir.dt.float32r
    AF = mybir.ActivationFunctionType

    nc = bacc.Bacc("TRN2", target_bir_lowering=False, debug=False)

    # float32r end-to-end: same bits as fp32 (PE rounds internally), but it
    # satisfies the BIR verifier's "rounded to FP32r" producer rule and runs
    # the PE at full rate (1 cyc/row at N>=256) instead of fp32's 4 cyc/row.
    xT = nc.dram_tensor("xT", [E, S], f32r, kind="ExternalInput")    # x[b].T
    wkT = nc.dram_tensor("wkT", [E, E], f32r, kind="ExternalInput")  # W_k.T
    wqT = nc.dram_tensor("wqT", [E, E], f32r, kind="ExternalInput")  # W_q.T
    mT = nc.dram_tensor("mT", [S, S], f32r, kind="ExternalInput")    # blur M.T
    out = nc.dram_tensor("out", [H, S, S], f32, kind="ExternalOutput")

    with tile.TileContext(nc) as tc:
        with (
            tc.tile_pool(name="persist", bufs=1) as pp,
            tc.tile_pool(name="qkb", bufs=2) as qp,
            tc.tile_pool(name="work", bufs=4) as wp,
            tc.tile_pool(name="stat", bufs=8) as sp,
            tc.tile_pool(name="pa", bufs=3, space="PSUM") as pa,
            tc.tile_pool(name="plg", bufs=2, space="PSUM") as plg,
        ):
            # ---- stage inputs in SBUF
            xts = []
            for i in range(NE):
                t = pp.tile([128, S], f32r, tag=f"x{i}", name=f"x{i}")
                nc.sync.dma_start(t[:], xT[i * 128:(i + 1) * 128, :])
                xts.append(t)
            wkts = []
            for i in range(NE):
                t = pp.tile([128, E], f32r, tag=f"wk{i}", name=f"wk{i}")
                nc.sync.dma_start(t[:], wkT[i * 128:(i + 1) * 128, :])
                wkts.append(t)
            wqts = []
            for i in range(NE):
                t = pp.tile([128, E], f32r, tag=f"wq{i}", name=f"wq{i}")
                nc.sync.dma_start(t[:], wqT[i * 128:(i + 1) * 128, :])
                wqts.append(t)
            mts = []
            for i in range(NS):
                t = pp.tile([128, S], f32r, tag=f"m{i}", name=f"m{i}")
                nc.sync.dma_start(t[:], mT[i * 128:(i + 1) * 128, :])
                mts.append(t)

            # ---- k = x @ Wk^T, laid out [s-tile partitions, f free]
            kts = []
            for st in range(NS):
                t = pp.tile([128, E], f32r, tag=f"k{st}", name=f"k{st}")
                kts.append(t)
            ncopy = 0
            for st in range(NS):
                for fb, (f0, fn) in enumerate(((0, 512), (512, 256))):
                    ps = pa.tile([128, 512], f32, tag="pa", name=f"psk{st}_{fb}")
                    for et in range(NE):
                        nc.tensor.matmul(
                            ps[:, 0:fn],
                            xts[et][:, st * 128:(st + 1) * 128],
                            wkts[et][:, f0:f0 + fn],
                            start=(et == 0),
                            stop=(et == NE - 1),
                        )
                    if ncopy % 2 == 0:
                        nc.scalar.copy(kts[st][:, f0:f0 + fn], ps[:, 0:fn])
                    else:
                        nc.vector.tensor_copy(kts[st][:, f0:f0 + fn], ps[:, 0:fn])
                    ncopy += 1

            # ---- per f-tile: qT, kbT, then 2 heads of logits/softmax/out
            for ft in range(NF):
                qt = qp.tile([128, S], f32r, tag="qT", name=f"qT{ft}")
                for sb in range(2):
                    ps = pa.tile([128, 512], f32, tag="pa", name=f"psq{ft}_{sb}")
                    for et in range(NE):
                        nc.tensor.matmul(
                            ps[:],
                            wqts[et][:, ft * 128:(ft + 1) * 128],
                            xts[et][:, sb * 512:(sb + 1) * 512],
                            start=(et == 0),
                            stop=(et == NE - 1),
                        )
                    nc.vector.tensor_copy(qt[:, sb * 512:(sb + 1) * 512], ps[:])

                kbt = qp.tile([128, S], f32r, tag="kbT", name=f"kbT{ft}")
                for sb in range(2):
                    ps = pa.tile([128, 512], f32, tag="pa", name=f"psb{ft}_{sb}")
                    for st in range(NS):
                        nc.tensor.matmul(
                            ps[:],
                            kts[st][:, ft * 128:(ft + 1) * 128],
                            mts[st][:, sb * 512:(sb + 1) * 512],
                            start=(st == 0),
                            stop=(st == NS - 1),
                        )
                    nc.vector.tensor_copy(kbt[:, sb * 512:(sb + 1) * 512], ps[:])

                for hh in range(2):
                    h = 2 * ft + hh
                    off = hh * D
                    for qc in range(NS):
                        lg = plg.tile([128, S], f32, tag="lg", name=f"lg{h}_{qc}")
                        for kb in range(2):
                            nc.tensor.matmul(
                                lg[:, kb * 512:(kb + 1) * 512],
                                qt[off:off + D, qc * 128:(qc + 1) * 128],
                                kbt[off:off + D, kb * 512:(kb + 1) * 512],
                                start=True,
                                stop=True,
                            )
                        ex = wp.tile([128, S], f32, tag="exp", name=f"ex{h}_{qc}")
                        acc = sp.tile([128, 1], f32, tag="acc", name=f"ac{h}_{qc}")
                        nc.scalar.activation(
                            ex[:], lg[:], AF.Exp, scale=SCALE, accum_out=acc[:]
                        )
                        rs = sp.tile([128, 1], f32, tag="rs", name=f"rs{h}_{qc}")
                        nc.vector.reciprocal(rs[:], acc[:])
                        nc.vector.tensor_scalar_mul(ex[:], ex[:], rs[:])
                        nc.sync.dma_start(out[h, qc * 128:(qc + 1) * 128, :], ex[:])

    nc.compile()
    return nc


_CACHE = {}


def _get_nc():
    if "nc" not in _CACHE:
        _CACHE["nc"] = _build()
    return _CACHE["nc"]


def _make_in_maps(x, W_in):
    x = np.ascontiguousarray(np.asarray(x), dtype=np.float32)
    W_in = np.ascontiguousarray(np.asarray(W_in), dtype=np.float32)
    Bm = _blur_matrix_1d()
    M = np.kron(Bm, Bm).astype(np.float32)        # [s_out, s_in]
    mTn = np.ascontiguousarray(M.T)               # [s_in, s_out]
    wkTn = np.ascontiguousarray(W_in[E:2 * E, :].T)       # [E, E]
    wqTn = np.ascontiguousarray(W_in[2 * E:3 * E, :].T)   # [E, E]
    in_maps = []
    for b in range(N_CORES):
        in_maps.append(
            {
                "xT": np.ascontiguousarray(x[b].T),
                "wkT": wkTn,
                "wqT": wqTn,
                "mT": mTn,
            }
        )
    return in_maps


def _run(x, W_in, trace=False):
    from concourse.bass_utils import run_bass_kernel_spmd

    nc = _get_nc()
    in_maps = _make_in_maps(x, W_in)
    res = run_bass_kernel_spmd(nc, in_maps, list(range(N_CORES)), trace=trace)
    outs = [np.asarray(res.results[c]["out"]) for c in range(N_CORES)]
    full = np.concatenate(outs, axis=0)  # [B*H, S, S]
    return full, res


def kernel(x, W_in):
    full, _ = _run(x, W_in, trace=False)
    return full
